# revision 1
# baseline (speedup 1.0000x reference)
"""Trainium2 Bass kernel for DifferentiableDefocusRenderer.

Math (mirrors the reference):
  planes = linspace(0, 50, 32); per-plane depthwise Gaussian blur of
  sharp_image (separable, kernel k<=31, truncated+renormalized), output =
  per-pixel hard select of the blurred plane by CoC bucket.

Distribution: pure data parallel, 8 cores = (batch b in 0..3) x (H half).
Each core computes [3, 256, 512] of output for its (b, half).

Per-core pipeline (all-plane dense, PE-heavy, float32r matmuls):
  pass A (column conv, 8-plane quarters, M-packed):
      C[x, i, y] = sum_k X[k, x] * T1[k, (i,q)]   (role-swapped matmul:
      stationary = X y-window tile, moving = multi-plane Toeplitz T1)
  pass B (row conv, per plane, 3-window PSUM accumulation):
      Q_i[xo, y] = sum_k T2_i[k, xo] * C[k, i, y]
  select: copy_predicated(acc, mask_i, Q_i) with one-hot plane masks
      (disjoint -> no ordering constraints, single accumulator),
  then PE-transpose acc back to [y, x] and DMA out.
C quarters are double-buffered so pass A(q+1) overlaps pass B(q) and the
PE streams continuously (keeps the HAM clock-gate warm).
"""

import sys

import numpy as np
import ml_dtypes

sys.path.insert(0, "/opt/trn_rl_repo")

B, C, H, W = 4, 3, 512, 512
MAX_COC = 50.0
NPLANES = 32
HALF = 256          # output rows per core
YT = 64             # output rows per pass-A y-tile
NT = HALF // YT     # 4 y-tiles
NS = W // 128       # 4 x slices
NQ = 4              # plane quarters (8 planes each)
BF16 = ml_dtypes.bfloat16

_CACHE = {}


# ----------------------------------------------------------------------------
# host-side tables (exactly mirroring reference kernel construction)
# ----------------------------------------------------------------------------

def _gaussian_kernel_1d(coc_value):
    # mirrors reference._gaussian_kernel_np (1-D factor of the outer product)
    sigma = coc_value / 2.355
    k = int(2 * coc_value + 1)
    if k % 2 == 0:
        k += 1
    k = min(k, 31)
    coords = np.arange(k, dtype=np.float32) - (k // 2)
    g = np.exp(-coords ** 2 / (2.0 * sigma ** 2))
    g = g / g.sum()
    return g.astype(np.float32)  # [k]


def _plane_kernels():
    """g31[i] in R^31, centered; plane 0 = identity delta."""
    planes = np.linspace(0.0, MAX_COC, NPLANES, dtype=np.float32)
    g31 = np.zeros((NPLANES, 31), dtype=np.float32)
    for i in range(NPLANES):
        coc = float(planes[i])
        if coc < 0.5:
            g31[i, 15] = 1.0
        else:
            g = _gaussian_kernel_1d(coc)
            k = g.shape[0]
            off = (31 - k) // 2
            g31[i, off:off + k] = g
    return planes, g31


def _host_tables():
    planes, g31 = _plane_kernels()

    # T1[k, q4, il*64 + q] = g31[8*q4 + il][k - q - 17]
    t1 = np.zeros((128, NQ, 512), dtype=np.float32)
    for q4 in range(NQ):
        for il in range(8):
            for q in range(64):
                for k in range(max(0, q + 17), min(128, q + 48)):
                    t1[k, q4, il * 64 + q] = g31[8 * q4 + il, k - q - 17]

    # T2 main [k, i, m] = g31[i][k - m + 15]
    t2m = np.zeros((128, NPLANES, 128), dtype=np.float32)
    for k in range(128):
        lo = max(0, k - 15)
        hi = min(128, k + 16)
        for m in range(lo, hi):
            t2m[k, :, m] = g31[:, k - m + 15]
    # packed corner windows:
    #  rows [0,32)   ("right"): [kk, i, m] = g31[i][kk - m + 143], m>=113
    #  rows [64,128) ("left") : [kk, i, m] = g31[i][kk - m - 113], m<15
    t2lr = np.zeros((128, NPLANES, 128), dtype=np.float32)
    for kk in range(32):
        for m in range(113, 128):
            idx = kk - m + 143
            if 0 <= idx <= 30:
                t2lr[kk, :, m] = g31[:, idx]
    for kk in range(64, 128):
        for m in range(15):
            idx = kk - m - 113
            if 0 <= idx <= 30:
                t2lr[kk, :, m] = g31[:, idx]

    ident = np.eye(128, dtype=np.float32)
    return planes, t1, t2m, t2lr, ident


def _plane_index(coc):
    """Exact bucket index per pixel, replicating reference fp32 comparisons."""
    planes = np.linspace(0.0, MAX_COC, NPLANES, dtype=np.float32)
    bnd = ((planes[:-1] + planes[1:]) / np.float32(2.0)).astype(np.float32)
    coc = coc.astype(np.float32)
    p = np.zeros(coc.shape, dtype=np.int32)
    for i in range(NPLANES - 1):
        p += (coc > bnd[i]).astype(np.int32)
    return p  # [H, W] int in [0, 31]


# ----------------------------------------------------------------------------
# device program
# ----------------------------------------------------------------------------

def _build_program():
    import concourse.bacc as bacc
    import concourse.mybir as mybir
    import concourse.tile as tile

    dt = mybir.dt
    nc = bacc.Bacc("TRN2", target_bir_lowering=False,
                   debug=False, enable_asserts=False, num_devices=8)

    xin_d = nc.dram_tensor("xin", [C, NT, 128, 512], dt.float32r,
                           kind="ExternalInput")
    t1_d = nc.dram_tensor("t1", [128, NQ, 512], dt.float32r,
                          kind="ExternalInput")
    t2m_d = nc.dram_tensor("t2m", [128, NPLANES, 128], dt.float32r,
                           kind="ExternalInput")
    t2lr_d = nc.dram_tensor("t2lr", [128, NPLANES, 128], dt.float32r,
                            kind="ExternalInput")
    pmap_d = nc.dram_tensor("pmap", [128, NS, HALF], dt.bfloat16,
                            kind="ExternalInput")
    id_d = nc.dram_tensor("ident", [128, 128], dt.float32,
                          kind="ExternalInput")
    out_d = nc.dram_tensor("out", [C, 2, 128, 512], dt.float32,
                           kind="ExternalOutput")

    with tile.TileContext(nc) as tc:
        with (
            tc.tile_pool(name="const", bufs=1) as const_pool,
            tc.tile_pool(name="cbuf", bufs=2) as c_pool,
            tc.tile_pool(name="xin", bufs=2) as x_pool,
            tc.tile_pool(name="work", bufs=2) as w_pool,
            tc.tile_pool(name="accp", bufs=1) as acc_pool,
            tc.tile_pool(name="psA", bufs=3, space="PSUM") as psA,
            tc.tile_pool(name="psB", bufs=2, space="PSUM") as psB,
            tc.tile_pool(name="psT", bufs=1, space="PSUM") as psT,
        ):
            # ---- constants ----
            t1_s = const_pool.tile([128, NQ, 512], dt.float32r, tag="t1")
            nc.sync.dma_start(t1_s[:], t1_d.ap()[:])
            t2m_s = const_pool.tile([128, NPLANES, 128], dt.float32r,
                                    tag="t2m")
            nc.sync.dma_start(t2m_s[:], t2m_d.ap()[:])
            t2lr_s = const_pool.tile([128, NPLANES, 128], dt.float32r,
                                     tag="t2lr")
            nc.sync.dma_start(t2lr_s[:], t2lr_d.ap()[:])
            pmap_s = const_pool.tile([128, NS, HALF], dt.bfloat16, tag="pmap")
            nc.sync.dma_start(pmap_s[:], pmap_d.ap()[:])
            id_s = const_pool.tile([128, 128], dt.float32, tag="ident")
            nc.sync.dma_start(id_s[:], id_d.ap()[:])

            masks = {}

            for ch in range(C):
                acc = acc_pool.tile([128, NS, HALF], dt.float32,
                                    tag="acc", name="acc")
                nc.gpsimd.memset(acc[:], 0.0)

                xts = []
                for t in range(NT):
                    xt = x_pool.tile([128, 512], dt.float32r, tag=f"xt{t}",
                                     name=f"xt{t}")
                    nc.sync.dma_start(xt[:], xin_d.ap()[ch, t])
                    xts.append(xt)

                for q4 in range(NQ):
                    # ---- pass A quarter: C[x, il, y] ----
                    c_all = c_pool.tile([128, NS, 8, HALF], dt.float32r,
                                        tag="c", name="c_all")
                    for t in range(NT):
                        for s in range(NS):
                            pa = psA.tile([128, 512], dt.float32, tag="pa",
                                          name="pa")
                            nc.tensor.matmul(
                                pa[:], xts[t][:, 128 * s:128 * (s + 1)],
                                t1_s[:, q4, :], start=True, stop=True)
                            # psum [x, (i8,q64)] -> C[x, il, 64t+q]
                            y0 = YT * t
                            if (t + s) % 2 == 0:
                                nc.scalar.copy(
                                    c_all[:, s, :, y0:y0 + YT],
                                    pa.rearrange("p (i q) -> p i q", i=8))
                            else:
                                nc.vector.tensor_copy(
                                    c_all[:, s, :, y0:y0 + YT],
                                    pa.rearrange("p (i q) -> p i q", i=8))

                    if ch == 0 and q4 == 0:
                        # one-hot plane masks (shared across channels); built
                        # here so they don't block the first pass A
                        for i in range(NPLANES):
                            mk = const_pool.tile([128, NS, HALF], dt.uint8,
                                                 tag=f"mask{i}",
                                                 name=f"mask{i}")
                            nc.vector.tensor_scalar(
                                mk[:], pmap_s[:],
                                float(i), None,
                                mybir.AluOpType.is_equal)
                            masks[i] = mk

                    # ---- pass B + select for this quarter ----
                    for g2 in range(2):
                        for j in range(4):
                            il = 4 * g2 + j
                            i = 8 * q4 + il
                            pb = psB.tile([128, NS, HALF], dt.float32,
                                          tag="pb", name="pb")
                            # merged MMs; out regions stay within one PSUM
                            # bank (bank A = s 0,1; bank B = s 2,3)
                            nc.tensor.matmul(            # mains, bank A
                                pb[:, 0:2, :], t2m_s[:, i, :],
                                c_all[:, 0:2, il, :], start=True, stop=False)
                            nc.tensor.matmul(            # mains, bank B
                                pb[:, 2:4, :], t2m_s[:, i, :],
                                c_all[:, 2:4, il, :], start=True, stop=False)
                            nc.tensor.matmul(            # left, out s=1
                                pb[:, 1, :], t2lr_s[64:128, i, :],
                                c_all[64:128, 0, il, :],
                                start=False, stop=False)
                            nc.tensor.matmul(            # left, out s=2,3
                                pb[:, 2:4, :], t2lr_s[64:128, i, :],
                                c_all[64:128, 1:3, il, :],
                                start=False, stop=False)
                            nc.tensor.matmul(            # right, out s=0,1
                                pb[:, 0:2, :], t2lr_s[0:32, i, :],
                                c_all[0:32, 1:3, il, :],
                                start=False, stop=True)
                            nc.tensor.matmul(            # right, out s=2
                                pb[:, 2, :], t2lr_s[0:32, i, :],
                                c_all[0:32, 3, il, :],
                                start=False, stop=True)
                            nc.vector.copy_predicated(
                                acc[:], masks[i][:], pb[:])

                # ---- transpose to [y, x], store ----
                for u in range(2):
                    onat = w_pool.tile([128, 512], dt.float32, tag=f"onat{u}",
                                       name=f"onat{u}")
                    for s in range(NS):
                        tp = psT.tile([128, 128], dt.float32, tag="tp")
                        nc.tensor.transpose(
                            tp[:], acc[:, s, 128 * u:128 * (u + 1)], id_s[:])
                        nc.scalar.copy(onat[:, 128 * s:128 * (s + 1)], tp[:])
                    nc.sync.dma_start(out_d.ap()[ch, u], onat[:])

    nc.compile()
    return nc


# ----------------------------------------------------------------------------
# host orchestration
# ----------------------------------------------------------------------------

def _prepare_in_maps(sharp_image, coc_map):
    planes, t1, t2m, t2lr, ident = _CACHE["tables"]
    p_full = {}
    in_maps = []
    for core in range(8):
        b, h = divmod(core, 2)
        y0 = HALF * h
        # X padded rows [-32, 288) local
        xpad = np.zeros((C, HALF + 64, W), dtype=np.float32)
        glo = y0 - 32
        ghi = y0 + HALF + 32
        clo, chi = max(0, glo), min(H, ghi)
        xpad[:, clo - glo:chi - glo, :] = sharp_image[b, :, clo:chi, :]
        xin = np.zeros((C, NT, 128, W), dtype=np.float32)
        for t in range(NT):
            xin[:, t] = xpad[:, YT * t:YT * t + 128, :]

        if b not in p_full:
            p_full[b] = _plane_index(coc_map[b, 0])
        p = p_full[b][y0:y0 + HALF, :]  # [HALF, W]
        # pmap[m, s, y] = p[y, 128s + m]
        pmap = np.ascontiguousarray(
            p.T.reshape(NS, 128, HALF).transpose(1, 0, 2)).astype(BF16)

        in_maps.append({
            "xin": xin,
            "t1": t1, "t2m": t2m, "t2lr": t2lr,
            "pmap": pmap, "ident": ident,
        })
    return in_maps


def _assemble(results):
    out = np.zeros((B, C, H, W), dtype=np.float32)
    for core in range(8):
        b, h = divmod(core, 2)
        r = results[core]["out"]  # [C, 2, 128, 512]
        out[b, :, HALF * h:HALF * (h + 1), :] = r.reshape(C, HALF, W)
    return out


def run(inputs, trace=False):
    from concourse import bass_utils
    if "tables" not in _CACHE:
        _CACHE["tables"] = _host_tables()
    if "nc" not in _CACHE:
        _CACHE["nc"] = _build_program()
    nc = _CACHE["nc"]
    in_maps = _prepare_in_maps(inputs["sharp_image"], inputs["coc_map"])
    res = bass_utils.run_bass_kernel_spmd(
        nc, in_maps, core_ids=list(range(8)), trace=trace)
    return _assemble(res.results), res


def kernel(**inputs):
    out, _ = run(inputs)
    return out

